# revision 32
# baseline (speedup 1.0000x reference)
"""Trainium2 Bass kernel for nn_Decision_Node (Linear+Hardtanh -> sp, 2-class
softmax Gini -> gini), data-parallel over 8 NeuronCores.

Math per core shard (B_s=128 of B=1024 batches, T=128, F=784, L=256, C=2):
    sp   = clip(x @ W.T + b, -1, 1)                      [N=16384, 256]
    p0   = sigmoid(sp * d),  d = contrib[...,0]-contrib[...,1]
    gini = 2 - p0^2 - p1^2 = 1.5 - 0.5*tanh(sp*d/2)^2

Device strategy (flipped layout: L on partitions, rows on free dim):
  - Stationary operand = W chunks [K=128, M=128]; moving operand =
    transposed-x tiles [K, N=512]. Bias folded as the 17th row of the
    last (K=17) contraction tile. 8 PSUM banks cycle the k=0..6
    accumulation so the PE never idles (no HAM re-throttle).
  - x is staged in DRAM chunk-major ([128, 6, ch] contiguous per chunk)
    so each chunk is ONE big DMA per queue half (24 KiB/partition lines),
    split across the sync + gpsimd queues; outputs ride the scalar queue.
  - DVE: fused hardtanh clip (PSUM drain), z = sp*d, sp uint8 quantize.
  - ACT: tanh(z/2); Square(sqrt(255)*th) -> u8 gini in one op.
  - Outputs u8, combined in one [128, (kind,lc,ch)] staging tile ->
    single DMA per chunk; host de-quantizes/transposes.
"""

import os
import sys
import types
from concurrent.futures import ThreadPoolExecutor

import numpy as np

for _p in (
    "/opt/trn_rl_repo",
    "/root/.axon_site",
    "/root/.axon_site/_ro/trn_rl_repo",
    "/root/.axon_site/_ro/pypackages",
):
    if os.path.isdir(_p) and _p not in sys.path:
        sys.path.append(_p)

B, T, F, L = 1024, 128, 784, 256
NCORES = 8
BS = B // NCORES          # batches per core
NROWS = BS * T            # 16384 rows per core
KT = 7                    # contraction tiles (784 = 6*128 + 16, + bias row)
KP = 17                   # contraction rows in the last k-tile (16 + bias)
CH = 2048                 # max rows per pipeline chunk
BANK = 512                # rows per PSUM bank / matmul free size
CHUNKS = (512, 1024, 1536) + (2048,) * 5 + (1024,) + (512,) * 4
FILLERS = {0: 2, 1: 6, 2: 2}  # post-chunk PE filler matmuls to bridge DMA ramp


def _build_module():
    import concourse.tile as tile
    from concourse import bacc, mybir

    f32, f16, u8 = mybir.dt.float32, mybir.dt.float16, mybir.dt.uint8
    Alu = mybir.AluOpType
    Act = mybir.ActivationFunctionType

    nc = bacc.Bacc(
        "TRN2",
        target_bir_lowering=False,
        debug=False,
        enable_asserts=False,
        num_devices=NCORES,
    )
    xt_d = nc.dram_tensor("xt", [6, 128, NROWS], f16, kind="ExternalInput").ap()
    # k6 remainder+bias rows, strip-packed per chunk: [32*bi + r, ci*BANK + j]
    x6_d = nc.dram_tensor(
        "x6", [128, len(CHUNKS) * BANK], f16, kind="ExternalInput"
    ).ap()
    wt_d = nc.dram_tensor("wt", [KT, 128, L], f16, kind="ExternalInput").ap()
    dr_d = nc.dram_tensor("dr", [2, 128, T], f16, kind="ExternalInput").ap()
    w6_d = nc.dram_tensor("w6", [128, 2, 128], f16, kind="ExternalInput").ap()
    # combined u8 outputs, chunk-major contiguous per partition:
    # [l, 4*n0 + (kind*2 + lc)*ch + j]
    oq_d = nc.dram_tensor("oq", [128, 4 * NROWS], u8, kind="ExternalOutput").ap()

    SQ255 = float(np.float32(np.sqrt(255.0)))

    with tile.TileContext(nc) as tc:
        with (
            tc.tile_pool(name="consts", bufs=1) as consts,
            tc.tile_pool(name="xt", bufs=3) as xt_pool,
            tc.tile_pool(name="psum", bufs=8, space="PSUM") as psum_pool,
            tc.tile_pool(name="sp", bufs=2) as sp_pool,
            tc.tile_pool(name="tmp", bufs=2) as tmp_pool,
            tc.tile_pool(name="outq", bufs=3) as outq_pool,
        ):
            wt_sb = consts.tile([128, KT, L], f16)
            nc.scalar.dma_start(wt_sb[:], wt_d.rearrange("k p l -> p k l"))
            w6_sb = consts.tile([128, 2, 128], f16, tag="w6")
            nc.scalar.dma_start(w6_sb[:], w6_d[:])
            dr_sb = consts.tile([128, 2, CH], f16)
            nc.scalar.dma_start(dr_sb[:, :, 0:T], dr_d.rearrange("c p n -> p c n"))
            # replicate d along the row axis: [*, lc, 0:128] -> [*, lc, 0:2048]
            w = T
            while w < CH:
                for lc in range(2):
                    nc.vector.tensor_scalar(
                        dr_sb[:, lc, w : 2 * w], dr_sb[:, lc, 0:w], 0.0, None, Alu.add
                    )
                w *= 2

            # PE warmup during the initial DMA wait so the HAM clock gate
            # flips to 8/8 right as real work arrives.
            wrm = consts.tile([128, BANK], f16, tag="wrm")
            nc.vector.memset(wrm[:], 0.0)
            b128 = consts.tile([128, 1], f32, tag="b128")
            nc.vector.memset(b128[:], 128.0)
            wps = psum_pool.tile([128, 2 * BANK], f32, tag="ps", bufs=4)
            for _ in range(30):
                nc.tensor.matmul(
                    wps[:, 0:BANK], wrm[:, 0:128], wrm[:], start=True, stop=True
                )
            wsink = consts.tile([128, 1], f16, tag="wsink")
            nc.vector.tensor_scalar(wsink[:], wps[:, 0:1], 0.0, None, Alu.mult)

            n0 = 0
            for ci, ch in enumerate(CHUNKS):
                nb = ch // BANK
                xks = []
                for k in range(6):
                    xk = xt_pool.tile([128, CH], f16, tag=f"x{k}", bufs=4)
                    eng = (nc.sync, nc.gpsimd, nc.sync, nc.gpsimd, nc.scalar, nc.gpsimd)[k]
                    eng.dma_start(xk[:, 0:ch], xt_d[k, :, n0 : n0 + ch])
                    xks.append(xk)
                x6 = xt_pool.tile([128, BANK], f16, tag="x6", bufs=4)
                nc.gpsimd.dma_start(
                    x6[:], x6_d[:, ci * BANK : (ci + 1) * BANK]
                )


                sp16 = sp_pool.tile([128, 2, CH], f16, tag="sp", bufs=2)
                for lc in range(2):
                    pairs = []
                    for bp in range((nb + 1) // 2):
                        pt = psum_pool.tile(
                            [128, 2 * BANK], f32, tag="ps", bufs=4, name=f"pp{bp}"
                        )
                        pairs.append(pt)
                    # ramp chunks run k-outer so each k-tile's matmuls
                    # start as soon as that tile's DMA lands
                    order = (
                        [(k, bi) for k in range(6) for bi in range(nb)]
                        if ci <= 3
                        else [(k, bi) for bi in range(nb) for k in range(6)]
                    )
                    for k, bi in order:
                        pss = pairs[bi // 2][:, (bi % 2) * BANK : (bi % 2 + 1) * BANK]
                        bb = bi * BANK
                        nc.tensor.matmul(
                            pss,
                            wt_sb[:, k, lc * 128 : (lc + 1) * 128],
                            xks[k][:, bb : bb + BANK],
                            start=(k == 0),
                            stop=False,
                        )
                    # k6 + bias: nb concurrent K=17 matmuls packed into one
                    # array pass via 32-row groups (tile_position)
                    for bi in range(nb):
                        pss = pairs[bi // 2][:, (bi % 2) * BANK : (bi % 2 + 1) * BANK]
                        nc.tensor.matmul(
                            pss,
                            w6_sb[32 * bi : 32 * bi + KP, lc, :],
                            x6[32 * bi : 32 * bi + KP, :],
                            start=False,
                            stop=True,
                            tile_position=(32 * bi, 0),
                        )
                    for bp in range((nb + 1) // 2):
                        nsub = min(2, nb - 2 * bp)
                        # fused hardtanh: (ps max -1) min 1, PSUM -> SBUF f16
                        nc.vector.tensor_scalar(
                            sp16[:, lc, 2 * bp * BANK : (2 * bp + nsub) * BANK],
                            pairs[bp][:, 0 : nsub * BANK],
                            -1.0,
                            1.0,
                            Alu.max,
                            Alu.min,
                        )
                z = tmp_pool.tile([128, 2, CH], f16, tag="z", bufs=2)
                th = tmp_pool.tile([128, 2, CH], f16, tag="th", bufs=2)
                oq = outq_pool.tile([128, 2, 2, CH], u8, tag="oq", bufs=3)
                tail = ci >= len(CHUNKS) - 2
                last = ci == len(CHUNKS) - 1
                nc.vector.tensor_tensor(
                    z[:, :, 0:ch], sp16[:, :, 0:ch], dr_sb[:, :, 0:ch], Alu.mult
                )
                nc.scalar.activation(th[:, :, 0:ch], z[:, :, 0:ch], Act.Tanh, scale=0.5)
                nc.vector.tensor_scalar(
                    oq[:, 0, :, 0:ch], sp16[:, :, 0:ch], 127.5, 128.0, Alu.mult, Alu.add
                )
                if not tail:
                    nc.scalar.activation(
                        oq[:, 1, :, 0:ch], th[:, :, 0:ch], Act.Square, scale=SQ255
                    )
                    nc.scalar.dma_start(
                        oq_d[:, 4 * n0 : 4 * n0 + 4 * ch].rearrange(
                            "p (a c j) -> p a c j", a=2, c=2
                        ),
                        oq[:, :, :, 0:ch],
                    )
                else:
                    # keep the tail chain short: square on DVE; final chunk's
                    # outs ride the now-idle sync (HWDGE) queue, split sp/gini
                    oeng = nc.sync if last else nc.scalar
                    oeng.dma_start(
                        oq_d[:, 4 * n0 : 4 * n0 + 2 * ch].rearrange(
                            "p (c j) -> p c j", c=2
                        ),
                        oq[:, 0, :, 0:ch],
                    )
                    th2 = tmp_pool.tile([128, 2, 2 * BANK], f16, tag="th2", bufs=2)
                    nc.vector.tensor_tensor(
                        th2[:, :, 0:ch], th[:, :, 0:ch], th[:, :, 0:ch], Alu.mult
                    )
                    nc.vector.tensor_scalar(
                        oq[:, 1, :, 0:ch], th2[:, :, 0:ch], 255.0, 0.5, Alu.mult, Alu.add
                    )
                    oeng.dma_start(
                        oq_d[:, 4 * n0 + 2 * ch : 4 * n0 + 4 * ch].rearrange(
                            "p (c j) -> p c j", c=2
                        ),
                        oq[:, 1, :, 0:ch],
                    )
                for _ in range(FILLERS.get(ci, 0)):
                    fps = psum_pool.tile([128, 2 * BANK], f32, tag="ps", bufs=4)
                    nc.tensor.matmul(
                        fps[:, 0:BANK], wrm[:, 0:128], wrm[:], start=True, stop=True
                    )
                n0 += ch

    nc.compile()
    return nc


def _prep_core_x(x_flat_core):
    """[16384, 784] fp32 -> (xt [6,128,n] f16, x6 strips [128, n//4]).

    x6 strip layout: rows 32*i..32*i+15 hold features 768..783 of bank i
    within each chunk; row 32*i+16 is the all-ones bias-fold row.
    """
    n = x_flat_core.shape[0]
    xsT16 = x_flat_core.T.astype(np.float16)  # [784, n], one strided pass
    xt = np.ascontiguousarray(xsT16[:768].reshape(6, 128, n))
    x6 = np.zeros((128, len(CHUNKS) * BANK), np.float16)
    n0 = 0
    for ci, ch in enumerate(CHUNKS):
        nb = ch // BANK
        blk = xsT16[768:784, n0 : n0 + ch].reshape(16, nb, BANK)
        for i in range(nb):
            x6[32 * i : 32 * i + 16, ci * BANK : (ci + 1) * BANK] = blk[:, i]
            x6[32 * i + 16, ci * BANK : (ci + 1) * BANK] = 1.0
        n0 += ch
    return xt, x6


def _prep_wt(W, b):
    wt = np.zeros((KT, 128, L), np.float16)
    WT = W.T  # [784, 256]
    for k in range(6):
        wt[k] = WT[k * 128 : (k + 1) * 128]
    wt[6, :16] = WT[768:784]
    wt[6, 16] = b
    w6 = np.zeros((128, 2, 128), np.float16)
    for i in range(4):
        w6[32 * i : 32 * i + 16] = WT[768:784].reshape(16, 2, 128)
        w6[32 * i + 16] = b.reshape(2, 128)
    return wt, w6


_module_cache = {}


def _get_module():
    if "m" not in _module_cache:
        _module_cache["m"] = _build_module()
    return _module_cache["m"]


def _install_ntff_hook():
    """Register the axon NTFF profiling hook missing from this image's antenv."""
    try:
        import antenv.axon_hooks  # noqa: F401

        return
    except ImportError:
        pass
    try:
        from trn_agent_boot.trn_boot import _ntff_profile_via_ctypes

        hook = _ntff_profile_via_ctypes("/opt/axon/libaxon_pjrt.so")
    except Exception:
        hook = None
    mod = types.ModuleType("antenv.axon_hooks")
    mod.get_axon_ntff_profile_hook = lambda: hook
    mod.set_axon_ntff_profile_hook = lambda h: None
    sys.modules["antenv.axon_hooks"] = mod


def _unstage(oq_raw):
    """[128, 4*16384] u8 chunk-major -> (sp, gini) [16384, 256] fp32."""
    spq_l = np.empty((2, 128, NROWS), np.uint8)
    giq_l = np.empty((2, 128, NROWS), np.uint8)
    n0 = 0
    for ch in CHUNKS:
        blk = oq_raw[:, 4 * n0 : 4 * n0 + 4 * ch].reshape(128, 2, 2, ch)
        spq_l[:, :, n0 : n0 + ch] = blk[:, 0].transpose(1, 0, 2)
        giq_l[:, :, n0 : n0 + ch] = blk[:, 1].transpose(1, 0, 2)
        n0 += ch
    spq = np.ascontiguousarray(spq_l.transpose(2, 0, 1).reshape(NROWS, L))
    giq = np.ascontiguousarray(giq_l.transpose(2, 0, 1).reshape(NROWS, L))
    sp = spq.astype(np.float32)
    sp -= 127.5
    sp *= 1.0 / 127.5
    gini = giq.astype(np.float32)
    gini *= -0.5 / 255.0
    gini += 1.5
    return sp, gini


def _run(x, W, b, contribution, trace=False, tmpdir=None):
    from concourse import bass_utils

    nc = _get_module()

    x_flat = np.ascontiguousarray(x, dtype=np.float32).reshape(NCORES, NROWS, F)
    wt, w6 = _prep_wt(np.asarray(W, np.float32), np.asarray(b, np.float32))
    c = np.asarray(contribution, np.float32)
    d = np.ascontiguousarray(c[:, :, 0] - c[:, :, 1], dtype=np.float32)
    dr = np.ascontiguousarray(d.T.astype(np.float16).reshape(2, 128, T))

    with ThreadPoolExecutor(NCORES) as ex:
        xs = list(ex.map(_prep_core_x, [x_flat[i] for i in range(NCORES)]))

    if trace:
        _install_ntff_hook()
    in_maps = [
        {"xt": xs[i][0], "x6": xs[i][1], "wt": wt, "dr": dr, "w6": w6}
        for i in range(NCORES)
    ]
    res = bass_utils.run_bass_kernel_spmd(
        nc, in_maps, core_ids=list(range(NCORES)), trace=trace, tmpdir=tmpdir
    )

    with ThreadPoolExecutor(NCORES) as ex:
        outs = list(ex.map(lambda i: _unstage(res.results[i]["oq"]), range(NCORES)))
    sp = np.concatenate([o[0] for o in outs]).reshape(B, T, L)
    gini = np.concatenate([o[1] for o in outs]).reshape(B, T, L)
    out = (sp, gini)
    return (out, res) if trace else (out, None)


def kernel(x, W, b, contribution):
    out, _ = _run(x, W, b, contribution, trace=False)
    return out


# revision 33
# speedup vs baseline: 1.0207x; 1.0207x over previous
"""Trainium2 Bass kernel for nn_Decision_Node (Linear+Hardtanh -> sp, 2-class
softmax Gini -> gini), data-parallel over 8 NeuronCores.

Math per core shard (B_s=128 of B=1024 batches, T=128, F=784, L=256, C=2):
    sp   = clip(x @ W.T + b, -1, 1)                      [N=16384, 256]
    p0   = sigmoid(sp * d),  d = contrib[...,0]-contrib[...,1]
    gini = 2 - p0^2 - p1^2 = 1.5 - 0.5*tanh(sp*d/2)^2

Device strategy (flipped layout: L on partitions, rows on free dim):
  - Stationary operand = W chunks [K=128, M=128]; moving operand =
    transposed-x tiles [K, N=512]. Bias folded as the 17th row of the
    last (K=17) contraction tile. 8 PSUM banks cycle the k=0..6
    accumulation so the PE never idles (no HAM re-throttle).
  - x is staged in DRAM chunk-major ([128, 6, ch] contiguous per chunk)
    so each chunk is ONE big DMA per queue half (24 KiB/partition lines),
    split across the sync + gpsimd queues; outputs ride the scalar queue.
  - DVE: fused hardtanh clip (PSUM drain), z = sp*d, sp uint8 quantize.
  - ACT: tanh(z/2); Square(sqrt(255)*th) -> u8 gini in one op.
  - Outputs u8, combined in one [128, (kind,lc,ch)] staging tile ->
    single DMA per chunk; host de-quantizes/transposes.
"""

import os
import sys
import types
from concurrent.futures import ThreadPoolExecutor

import numpy as np

for _p in (
    "/opt/trn_rl_repo",
    "/root/.axon_site",
    "/root/.axon_site/_ro/trn_rl_repo",
    "/root/.axon_site/_ro/pypackages",
):
    if os.path.isdir(_p) and _p not in sys.path:
        sys.path.append(_p)

B, T, F, L = 1024, 128, 784, 256
NCORES = 8
BS = B // NCORES          # batches per core
NROWS = BS * T            # 16384 rows per core
KT = 7                    # contraction tiles (784 = 6*128 + 16, + bias row)
KP = 17                   # contraction rows in the last k-tile (16 + bias)
CH = 2048                 # max rows per pipeline chunk
BANK = 512                # rows per PSUM bank / matmul free size
CHUNKS = (512, 1024, 1536) + (2048,) * 5 + (1024,) + (512,) * 4
FILLERS = {0: 4, 1: 30, 2: 2}  # post-chunk PE filler matmuls to bridge DMA ramp


def _build_module():
    import concourse.tile as tile
    from concourse import bacc, mybir

    f32, f16, u8 = mybir.dt.float32, mybir.dt.float16, mybir.dt.uint8
    Alu = mybir.AluOpType
    Act = mybir.ActivationFunctionType

    nc = bacc.Bacc(
        "TRN2",
        target_bir_lowering=False,
        debug=False,
        enable_asserts=False,
        num_devices=NCORES,
    )
    xt_d = nc.dram_tensor("xt", [6, 128, NROWS], f16, kind="ExternalInput").ap()
    # k6 remainder+bias rows, strip-packed per chunk: [32*bi + r, ci*BANK + j]
    x6_d = nc.dram_tensor(
        "x6", [128, len(CHUNKS) * BANK], f16, kind="ExternalInput"
    ).ap()
    wt_d = nc.dram_tensor("wt", [KT, 128, L], f16, kind="ExternalInput").ap()
    dr_d = nc.dram_tensor("dr", [2, 128, T], f16, kind="ExternalInput").ap()
    w6_d = nc.dram_tensor("w6", [128, 2, 128], f16, kind="ExternalInput").ap()
    # combined u8 outputs, chunk-major contiguous per partition:
    # [l, 4*n0 + (kind*2 + lc)*ch + j]
    oq_d = nc.dram_tensor("oq", [128, 4 * NROWS], u8, kind="ExternalOutput").ap()

    SQ255 = float(np.float32(np.sqrt(255.0)))

    with tile.TileContext(nc) as tc:
        with (
            tc.tile_pool(name="consts", bufs=1) as consts,
            tc.tile_pool(name="xt", bufs=3) as xt_pool,
            tc.tile_pool(name="psum", bufs=8, space="PSUM") as psum_pool,
            tc.tile_pool(name="sp", bufs=2) as sp_pool,
            tc.tile_pool(name="tmp", bufs=2) as tmp_pool,
            tc.tile_pool(name="outq", bufs=3) as outq_pool,
        ):
            wt_sb = consts.tile([128, KT, L], f16)
            nc.scalar.dma_start(wt_sb[:], wt_d.rearrange("k p l -> p k l"))
            w6_sb = consts.tile([128, 2, 128], f16, tag="w6")
            nc.scalar.dma_start(w6_sb[:], w6_d[:])
            dr_sb = consts.tile([128, 2, CH], f16)
            nc.scalar.dma_start(dr_sb[:, :, 0:T], dr_d.rearrange("c p n -> p c n"))
            # replicate d along the row axis: [*, lc, 0:128] -> [*, lc, 0:2048]
            w = T
            while w < CH:
                for lc in range(2):
                    nc.vector.tensor_scalar(
                        dr_sb[:, lc, w : 2 * w], dr_sb[:, lc, 0:w], 0.0, None, Alu.add
                    )
                w *= 2

            # PE warmup during the initial DMA wait so the HAM clock gate
            # flips to 8/8 right as real work arrives.
            wrm = consts.tile([128, BANK], f16, tag="wrm")
            nc.vector.memset(wrm[:], 0.0)
            b128 = consts.tile([128, 1], f32, tag="b128")
            nc.vector.memset(b128[:], 128.0)
            wps = psum_pool.tile([128, 2 * BANK], f32, tag="ps", bufs=4)
            for _ in range(30):
                nc.tensor.matmul(
                    wps[:, 0:BANK], wrm[:, 0:128], wrm[:], start=True, stop=True
                )
            wsink = consts.tile([128, 1], f16, tag="wsink")
            nc.vector.tensor_scalar(wsink[:], wps[:, 0:1], 0.0, None, Alu.mult)

            n0 = 0
            for ci, ch in enumerate(CHUNKS):
                nb = ch // BANK
                xks = []
                for k in range(6):
                    xk = xt_pool.tile([128, CH], f16, tag=f"x{k}", bufs=4)
                    eng = (nc.sync, nc.gpsimd, nc.sync, nc.gpsimd, nc.scalar, nc.gpsimd)[k]
                    eng.dma_start(xk[:, 0:ch], xt_d[k, :, n0 : n0 + ch])
                    xks.append(xk)
                x6 = xt_pool.tile([128, BANK], f16, tag="x6", bufs=4)
                nc.gpsimd.dma_start(
                    x6[:], x6_d[:, ci * BANK : (ci + 1) * BANK]
                )


                sp16 = sp_pool.tile([128, 2, CH], f16, tag="sp", bufs=2)
                for lc in range(2):
                    pairs = []
                    for bp in range((nb + 1) // 2):
                        pt = psum_pool.tile(
                            [128, 2 * BANK], f32, tag="ps", bufs=4, name=f"pp{bp}"
                        )
                        pairs.append(pt)
                    for bi in range(nb):
                        pss = pairs[bi // 2][:, (bi % 2) * BANK : (bi % 2 + 1) * BANK]
                        bb = bi * BANK
                        for k in range(6):
                            nc.tensor.matmul(
                                pss,
                                wt_sb[:, k, lc * 128 : (lc + 1) * 128],
                                xks[k][:, bb : bb + BANK],
                                start=(k == 0),
                                stop=False,
                            )
                    # k6 + bias: nb concurrent K=17 matmuls packed into one
                    # array pass via 32-row groups (tile_position)
                    for bi in range(nb):
                        pss = pairs[bi // 2][:, (bi % 2) * BANK : (bi % 2 + 1) * BANK]
                        nc.tensor.matmul(
                            pss,
                            w6_sb[32 * bi : 32 * bi + KP, lc, :],
                            x6[32 * bi : 32 * bi + KP, :],
                            start=False,
                            stop=True,
                            tile_position=(32 * bi, 0),
                        )
                    for bp in range((nb + 1) // 2):
                        nsub = min(2, nb - 2 * bp)
                        # fused hardtanh: (ps max -1) min 1, PSUM -> SBUF f16
                        nc.vector.tensor_scalar(
                            sp16[:, lc, 2 * bp * BANK : (2 * bp + nsub) * BANK],
                            pairs[bp][:, 0 : nsub * BANK],
                            -1.0,
                            1.0,
                            Alu.max,
                            Alu.min,
                        )
                z = tmp_pool.tile([128, 2, CH], f16, tag="z", bufs=2)
                th = tmp_pool.tile([128, 2, CH], f16, tag="th", bufs=2)
                oq = outq_pool.tile([128, 2, 2, CH], u8, tag="oq", bufs=3)
                tail = ci >= len(CHUNKS) - 2
                last = ci == len(CHUNKS) - 1
                nc.vector.tensor_tensor(
                    z[:, :, 0:ch], sp16[:, :, 0:ch], dr_sb[:, :, 0:ch], Alu.mult
                )
                nc.scalar.activation(th[:, :, 0:ch], z[:, :, 0:ch], Act.Tanh, scale=0.5)
                nc.vector.tensor_scalar(
                    oq[:, 0, :, 0:ch], sp16[:, :, 0:ch], 127.5, 128.0, Alu.mult, Alu.add
                )
                if not tail:
                    nc.scalar.activation(
                        oq[:, 1, :, 0:ch], th[:, :, 0:ch], Act.Square, scale=SQ255
                    )
                    nc.scalar.dma_start(
                        oq_d[:, 4 * n0 : 4 * n0 + 4 * ch].rearrange(
                            "p (a c j) -> p a c j", a=2, c=2
                        ),
                        oq[:, :, :, 0:ch],
                    )
                else:
                    # keep the tail chain short: square on DVE; final chunk's
                    # outs ride the now-idle sync (HWDGE) queue, split sp/gini
                    oeng = nc.sync if last else nc.scalar
                    oeng.dma_start(
                        oq_d[:, 4 * n0 : 4 * n0 + 2 * ch].rearrange(
                            "p (c j) -> p c j", c=2
                        ),
                        oq[:, 0, :, 0:ch],
                    )
                    th2 = tmp_pool.tile([128, 2, 2 * BANK], f16, tag="th2", bufs=2)
                    nc.vector.tensor_tensor(
                        th2[:, :, 0:ch], th[:, :, 0:ch], th[:, :, 0:ch], Alu.mult
                    )
                    nc.vector.tensor_scalar(
                        oq[:, 1, :, 0:ch], th2[:, :, 0:ch], 255.0, 0.5, Alu.mult, Alu.add
                    )
                    oeng.dma_start(
                        oq_d[:, 4 * n0 + 2 * ch : 4 * n0 + 4 * ch].rearrange(
                            "p (c j) -> p c j", c=2
                        ),
                        oq[:, 1, :, 0:ch],
                    )
                for _ in range(FILLERS.get(ci, 0)):
                    fps = psum_pool.tile([128, 2 * BANK], f32, tag="ps", bufs=4)
                    nc.tensor.matmul(
                        fps[:, 0:BANK], wrm[:, 0:128], wrm[:], start=True, stop=True
                    )
                n0 += ch

    nc.compile()
    return nc


def _prep_core_x(x_flat_core):
    """[16384, 784] fp32 -> (xt [6,128,n] f16, x6 strips [128, n//4]).

    x6 strip layout: rows 32*i..32*i+15 hold features 768..783 of bank i
    within each chunk; row 32*i+16 is the all-ones bias-fold row.
    """
    n = x_flat_core.shape[0]
    xsT16 = x_flat_core.T.astype(np.float16)  # [784, n], one strided pass
    xt = np.ascontiguousarray(xsT16[:768].reshape(6, 128, n))
    x6 = np.zeros((128, len(CHUNKS) * BANK), np.float16)
    n0 = 0
    for ci, ch in enumerate(CHUNKS):
        nb = ch // BANK
        blk = xsT16[768:784, n0 : n0 + ch].reshape(16, nb, BANK)
        for i in range(nb):
            x6[32 * i : 32 * i + 16, ci * BANK : (ci + 1) * BANK] = blk[:, i]
            x6[32 * i + 16, ci * BANK : (ci + 1) * BANK] = 1.0
        n0 += ch
    return xt, x6


def _prep_wt(W, b):
    wt = np.zeros((KT, 128, L), np.float16)
    WT = W.T  # [784, 256]
    for k in range(6):
        wt[k] = WT[k * 128 : (k + 1) * 128]
    wt[6, :16] = WT[768:784]
    wt[6, 16] = b
    w6 = np.zeros((128, 2, 128), np.float16)
    for i in range(4):
        w6[32 * i : 32 * i + 16] = WT[768:784].reshape(16, 2, 128)
        w6[32 * i + 16] = b.reshape(2, 128)
    return wt, w6


_module_cache = {}


def _get_module():
    if "m" not in _module_cache:
        _module_cache["m"] = _build_module()
    return _module_cache["m"]


def _install_ntff_hook():
    """Register the axon NTFF profiling hook missing from this image's antenv."""
    try:
        import antenv.axon_hooks  # noqa: F401

        return
    except ImportError:
        pass
    try:
        from trn_agent_boot.trn_boot import _ntff_profile_via_ctypes

        hook = _ntff_profile_via_ctypes("/opt/axon/libaxon_pjrt.so")
    except Exception:
        hook = None
    mod = types.ModuleType("antenv.axon_hooks")
    mod.get_axon_ntff_profile_hook = lambda: hook
    mod.set_axon_ntff_profile_hook = lambda h: None
    sys.modules["antenv.axon_hooks"] = mod


def _unstage(oq_raw):
    """[128, 4*16384] u8 chunk-major -> (sp, gini) [16384, 256] fp32."""
    spq_l = np.empty((2, 128, NROWS), np.uint8)
    giq_l = np.empty((2, 128, NROWS), np.uint8)
    n0 = 0
    for ch in CHUNKS:
        blk = oq_raw[:, 4 * n0 : 4 * n0 + 4 * ch].reshape(128, 2, 2, ch)
        spq_l[:, :, n0 : n0 + ch] = blk[:, 0].transpose(1, 0, 2)
        giq_l[:, :, n0 : n0 + ch] = blk[:, 1].transpose(1, 0, 2)
        n0 += ch
    spq = np.ascontiguousarray(spq_l.transpose(2, 0, 1).reshape(NROWS, L))
    giq = np.ascontiguousarray(giq_l.transpose(2, 0, 1).reshape(NROWS, L))
    sp = spq.astype(np.float32)
    sp -= 127.5
    sp *= 1.0 / 127.5
    gini = giq.astype(np.float32)
    gini *= -0.5 / 255.0
    gini += 1.5
    return sp, gini


def _run(x, W, b, contribution, trace=False, tmpdir=None):
    from concourse import bass_utils

    nc = _get_module()

    x_flat = np.ascontiguousarray(x, dtype=np.float32).reshape(NCORES, NROWS, F)
    wt, w6 = _prep_wt(np.asarray(W, np.float32), np.asarray(b, np.float32))
    c = np.asarray(contribution, np.float32)
    d = np.ascontiguousarray(c[:, :, 0] - c[:, :, 1], dtype=np.float32)
    dr = np.ascontiguousarray(d.T.astype(np.float16).reshape(2, 128, T))

    with ThreadPoolExecutor(NCORES) as ex:
        xs = list(ex.map(_prep_core_x, [x_flat[i] for i in range(NCORES)]))

    if trace:
        _install_ntff_hook()
    in_maps = [
        {"xt": xs[i][0], "x6": xs[i][1], "wt": wt, "dr": dr, "w6": w6}
        for i in range(NCORES)
    ]
    res = bass_utils.run_bass_kernel_spmd(
        nc, in_maps, core_ids=list(range(NCORES)), trace=trace, tmpdir=tmpdir
    )

    with ThreadPoolExecutor(NCORES) as ex:
        outs = list(ex.map(lambda i: _unstage(res.results[i]["oq"]), range(NCORES)))
    sp = np.concatenate([o[0] for o in outs]).reshape(B, T, L)
    gini = np.concatenate([o[1] for o in outs]).reshape(B, T, L)
    out = (sp, gini)
    return (out, res) if trace else (out, None)


def kernel(x, W, b, contribution):
    out, _ = _run(x, W, b, contribution, trace=False)
    return out


# revision 34
# speedup vs baseline: 1.0230x; 1.0023x over previous
"""Trainium2 Bass kernel for nn_Decision_Node (Linear+Hardtanh -> sp, 2-class
softmax Gini -> gini), data-parallel over 8 NeuronCores.

Math per core shard (B_s=128 of B=1024 batches, T=128, F=784, L=256, C=2):
    sp   = clip(x @ W.T + b, -1, 1)                      [N=16384, 256]
    p0   = sigmoid(sp * d),  d = contrib[...,0]-contrib[...,1]
    gini = 2 - p0^2 - p1^2 = 1.5 - 0.5*tanh(sp*d/2)^2

Device strategy (flipped layout: L on partitions, rows on free dim):
  - Stationary operand = W chunks [K=128, M=128]; moving operand =
    transposed-x tiles [K, N=512] streamed from SBUF; fp16 matmul with
    fp32 PSUM accumulation. Bias rides as the 17th contraction row of
    the last (K=17) k-tile. PSUM used as four [128, 1024] bank-pair
    tiles; per (chunk, lc) the 24 k0-5 matmuls accumulate bank-wise,
    then the K=17 remainder+bias matmuls for all 4 banks run as ONE
    concurrent quad via tile_position 32-row groups (saves ~11% PE).
  - PE kept continuously busy (HAM clock-gate stays 8/8 at 2.4 GHz, no
    P0 downclock): 30 warmup matmuls cover the initial DMA wait and
    filler matmuls sized to the HBM supply schedule bridge the ramp.
  - x k-tiles stream per-chunk on the sync/gpsimd/scalar queues (4 KiB
    per-partition lines saturate HBM at ~330-400 GB/s); k6 strips are
    host-packed at 32-partition offsets for the quad (one DMA/chunk).
  - DVE: fused hardtanh clip (PSUM drain, FD-1024 pairs), z = sp*d
    (d is L-major so it tiles along rows), sp uint8 quantize.
  - ACT: tanh(z/2); Square(sqrt(255)*th) -> u8 gini in one op (tail
    chunks square on DVE instead to shorten the drain chain).
  - Outputs u8 in one [128, (kind,lc,ch)] staging tile -> one DMA per
    chunk into a chunk-major contiguous DRAM layout (8 KiB lines);
    host de-quantizes/transposes.
"""

import os
import sys
import types
from concurrent.futures import ThreadPoolExecutor

import numpy as np

for _p in (
    "/opt/trn_rl_repo",
    "/root/.axon_site",
    "/root/.axon_site/_ro/trn_rl_repo",
    "/root/.axon_site/_ro/pypackages",
):
    if os.path.isdir(_p) and _p not in sys.path:
        sys.path.append(_p)

B, T, F, L = 1024, 128, 784, 256
NCORES = 8
BS = B // NCORES          # batches per core
NROWS = BS * T            # 16384 rows per core
KT = 7                    # contraction tiles (784 = 6*128 + 16, + bias row)
KP = 17                   # contraction rows in the last k-tile (16 + bias)
CH = 2048                 # max rows per pipeline chunk
BANK = 512                # rows per PSUM bank / matmul free size
CHUNKS = (512, 1024, 1536) + (2048,) * 5 + (1024,) + (512,) * 4
FILLERS = {0: 4, 1: 30, 2: 2}  # post-chunk PE filler matmuls to bridge DMA ramp


def _build_module():
    import concourse.tile as tile
    from concourse import bacc, mybir

    f32, f16, u8 = mybir.dt.float32, mybir.dt.float16, mybir.dt.uint8
    Alu = mybir.AluOpType
    Act = mybir.ActivationFunctionType

    nc = bacc.Bacc(
        "TRN2",
        target_bir_lowering=False,
        debug=False,
        enable_asserts=False,
        num_devices=NCORES,
    )
    xt_d = nc.dram_tensor("xt", [6, 128, NROWS], f16, kind="ExternalInput").ap()
    # k6 remainder+bias rows, strip-packed per chunk: [32*bi + r, ci*BANK + j]
    x6_d = nc.dram_tensor(
        "x6", [128, len(CHUNKS) * BANK], f16, kind="ExternalInput"
    ).ap()
    wt_d = nc.dram_tensor("wt", [KT, 128, L], f16, kind="ExternalInput").ap()
    dr_d = nc.dram_tensor("dr", [2, 128, T], f16, kind="ExternalInput").ap()
    w6_d = nc.dram_tensor("w6", [128, 2, 128], f16, kind="ExternalInput").ap()
    # combined u8 outputs, chunk-major contiguous per partition:
    # [l, 4*n0 + (kind*2 + lc)*ch + j]
    oq_d = nc.dram_tensor("oq", [128, 4 * NROWS], u8, kind="ExternalOutput").ap()

    SQ255 = float(np.float32(np.sqrt(255.0)))

    with tile.TileContext(nc) as tc:
        with (
            tc.tile_pool(name="consts", bufs=1) as consts,
            tc.tile_pool(name="xt", bufs=3) as xt_pool,
            tc.tile_pool(name="psum", bufs=8, space="PSUM") as psum_pool,
            tc.tile_pool(name="sp", bufs=2) as sp_pool,
            tc.tile_pool(name="tmp", bufs=2) as tmp_pool,
            tc.tile_pool(name="outq", bufs=3) as outq_pool,
        ):
            wt_sb = consts.tile([128, KT, L], f16)
            nc.scalar.dma_start(wt_sb[:], wt_d.rearrange("k p l -> p k l"))
            w6_sb = consts.tile([128, 2, 128], f16, tag="w6")
            nc.scalar.dma_start(w6_sb[:], w6_d[:])
            dr_sb = consts.tile([128, 2, CH], f16)
            nc.scalar.dma_start(dr_sb[:, :, 0:T], dr_d.rearrange("c p n -> p c n"))
            # replicate d along the row axis: [*, lc, 0:128] -> [*, lc, 0:2048]
            w = T
            while w < CH:
                for lc in range(2):
                    nc.vector.tensor_scalar(
                        dr_sb[:, lc, w : 2 * w], dr_sb[:, lc, 0:w], 0.0, None, Alu.add
                    )
                w *= 2

            # PE warmup during the initial DMA wait so the HAM clock gate
            # flips to 8/8 right as real work arrives.
            wrm = consts.tile([128, BANK], f16, tag="wrm")
            nc.vector.memset(wrm[:], 0.0)
            b128 = consts.tile([128, 1], f32, tag="b128")
            nc.vector.memset(b128[:], 128.0)
            wps = psum_pool.tile([128, 2 * BANK], f32, tag="ps", bufs=4)
            for _ in range(30):
                nc.tensor.matmul(
                    wps[:, 0:BANK], wrm[:, 0:128], wrm[:], start=True, stop=True
                )
            wsink = consts.tile([128, 1], f16, tag="wsink")
            nc.vector.tensor_scalar(wsink[:], wps[:, 0:1], 0.0, None, Alu.mult)

            n0 = 0
            for ci, ch in enumerate(CHUNKS):
                nb = ch // BANK
                xks = []
                for k in range(6):
                    xk = xt_pool.tile([128, CH], f16, tag=f"x{k}", bufs=4)
                    eng = (nc.sync, nc.gpsimd, nc.sync, nc.gpsimd, nc.scalar, nc.gpsimd)[k]
                    eng.dma_start(xk[:, 0:ch], xt_d[k, :, n0 : n0 + ch])
                    xks.append(xk)
                x6 = xt_pool.tile([128, BANK], f16, tag="x6", bufs=4)
                nc.gpsimd.dma_start(
                    x6[:], x6_d[:, ci * BANK : (ci + 1) * BANK]
                )


                sp16 = sp_pool.tile([128, 2, CH], f16, tag="sp", bufs=2)
                for lc in range(2):
                    pairs = []
                    for bp in range((nb + 1) // 2):
                        pt = psum_pool.tile(
                            [128, 2 * BANK], f32, tag="ps", bufs=4, name=f"pp{bp}"
                        )
                        pairs.append(pt)
                    for bi in range(nb):
                        pss = pairs[bi // 2][:, (bi % 2) * BANK : (bi % 2 + 1) * BANK]
                        bb = bi * BANK
                        for k in range(6):
                            nc.tensor.matmul(
                                pss,
                                wt_sb[:, k, lc * 128 : (lc + 1) * 128],
                                xks[k][:, bb : bb + BANK],
                                start=(k == 0),
                                stop=False,
                            )
                    # k6 + bias: nb concurrent K=17 matmuls packed into one
                    # array pass via 32-row groups (tile_position)
                    for bi in range(nb):
                        pss = pairs[bi // 2][:, (bi % 2) * BANK : (bi % 2 + 1) * BANK]
                        nc.tensor.matmul(
                            pss,
                            w6_sb[32 * bi : 32 * bi + KP, lc, :],
                            x6[32 * bi : 32 * bi + KP, :],
                            start=False,
                            stop=True,
                            tile_position=(32 * bi, 0),
                        )
                    for bp in range((nb + 1) // 2):
                        nsub = min(2, nb - 2 * bp)
                        # fused hardtanh: (ps max -1) min 1, PSUM -> SBUF f16
                        nc.vector.tensor_scalar(
                            sp16[:, lc, 2 * bp * BANK : (2 * bp + nsub) * BANK],
                            pairs[bp][:, 0 : nsub * BANK],
                            -1.0,
                            1.0,
                            Alu.max,
                            Alu.min,
                        )
                z = tmp_pool.tile([128, 2, CH], f16, tag="z", bufs=2)
                th = tmp_pool.tile([128, 2, CH], f16, tag="th", bufs=2)
                oq = outq_pool.tile([128, 2, 2, CH], u8, tag="oq", bufs=3)
                tail = ci >= len(CHUNKS) - 2
                last = ci == len(CHUNKS) - 1
                nc.vector.tensor_tensor(
                    z[:, :, 0:ch], sp16[:, :, 0:ch], dr_sb[:, :, 0:ch], Alu.mult
                )
                nc.scalar.activation(th[:, :, 0:ch], z[:, :, 0:ch], Act.Tanh, scale=0.5)
                nc.vector.tensor_scalar(
                    oq[:, 0, :, 0:ch], sp16[:, :, 0:ch], 127.5, 128.0, Alu.mult, Alu.add
                )
                if not tail:
                    nc.scalar.activation(
                        oq[:, 1, :, 0:ch], th[:, :, 0:ch], Act.Square, scale=SQ255
                    )
                    nc.scalar.dma_start(
                        oq_d[:, 4 * n0 : 4 * n0 + 4 * ch].rearrange(
                            "p (a c j) -> p a c j", a=2, c=2
                        ),
                        oq[:, :, :, 0:ch],
                    )
                else:
                    # keep the tail chain short: square on DVE; final chunk's
                    # outs ride the now-idle sync (HWDGE) queue, split sp/gini
                    oeng = nc.sync if last else nc.scalar
                    oeng.dma_start(
                        oq_d[:, 4 * n0 : 4 * n0 + 2 * ch].rearrange(
                            "p (c j) -> p c j", c=2
                        ),
                        oq[:, 0, :, 0:ch],
                    )
                    th2 = tmp_pool.tile([128, 2, 2 * BANK], f16, tag="th2", bufs=2)
                    nc.vector.tensor_tensor(
                        th2[:, :, 0:ch], th[:, :, 0:ch], th[:, :, 0:ch], Alu.mult
                    )
                    nc.vector.tensor_scalar(
                        oq[:, 1, :, 0:ch], th2[:, :, 0:ch], 255.0, 0.5, Alu.mult, Alu.add
                    )
                    oeng.dma_start(
                        oq_d[:, 4 * n0 + 2 * ch : 4 * n0 + 4 * ch].rearrange(
                            "p (c j) -> p c j", c=2
                        ),
                        oq[:, 1, :, 0:ch],
                    )
                for _ in range(FILLERS.get(ci, 0)):
                    fps = psum_pool.tile([128, 2 * BANK], f32, tag="ps", bufs=4)
                    nc.tensor.matmul(
                        fps[:, 0:BANK], wrm[:, 0:128], wrm[:], start=True, stop=True
                    )
                n0 += ch

    nc.compile()
    return nc


def _prep_core_x(x_flat_core):
    """[16384, 784] fp32 -> (xt [6,128,n] f16, x6 strips [128, n//4]).

    x6 strip layout: rows 32*i..32*i+15 hold features 768..783 of bank i
    within each chunk; row 32*i+16 is the all-ones bias-fold row.
    """
    n = x_flat_core.shape[0]
    xsT16 = x_flat_core.T.astype(np.float16)  # [784, n], one strided pass
    xt = np.ascontiguousarray(xsT16[:768].reshape(6, 128, n))
    x6 = np.zeros((128, len(CHUNKS) * BANK), np.float16)
    n0 = 0
    for ci, ch in enumerate(CHUNKS):
        nb = ch // BANK
        blk = xsT16[768:784, n0 : n0 + ch].reshape(16, nb, BANK)
        for i in range(nb):
            x6[32 * i : 32 * i + 16, ci * BANK : (ci + 1) * BANK] = blk[:, i]
            x6[32 * i + 16, ci * BANK : (ci + 1) * BANK] = 1.0
        n0 += ch
    return xt, x6


def _prep_wt(W, b):
    wt = np.zeros((KT, 128, L), np.float16)
    WT = W.T  # [784, 256]
    for k in range(6):
        wt[k] = WT[k * 128 : (k + 1) * 128]
    wt[6, :16] = WT[768:784]
    wt[6, 16] = b
    w6 = np.zeros((128, 2, 128), np.float16)
    for i in range(4):
        w6[32 * i : 32 * i + 16] = WT[768:784].reshape(16, 2, 128)
        w6[32 * i + 16] = b.reshape(2, 128)
    return wt, w6


_module_cache = {}


def _get_module():
    if "m" not in _module_cache:
        _module_cache["m"] = _build_module()
    return _module_cache["m"]


def _install_ntff_hook():
    """Register the axon NTFF profiling hook missing from this image's antenv."""
    try:
        import antenv.axon_hooks  # noqa: F401

        return
    except ImportError:
        pass
    try:
        from trn_agent_boot.trn_boot import _ntff_profile_via_ctypes

        hook = _ntff_profile_via_ctypes("/opt/axon/libaxon_pjrt.so")
    except Exception:
        hook = None
    mod = types.ModuleType("antenv.axon_hooks")
    mod.get_axon_ntff_profile_hook = lambda: hook
    mod.set_axon_ntff_profile_hook = lambda h: None
    sys.modules["antenv.axon_hooks"] = mod


def _unstage(oq_raw):
    """[128, 4*16384] u8 chunk-major -> (sp, gini) [16384, 256] fp32."""
    spq_l = np.empty((2, 128, NROWS), np.uint8)
    giq_l = np.empty((2, 128, NROWS), np.uint8)
    n0 = 0
    for ch in CHUNKS:
        blk = oq_raw[:, 4 * n0 : 4 * n0 + 4 * ch].reshape(128, 2, 2, ch)
        spq_l[:, :, n0 : n0 + ch] = blk[:, 0].transpose(1, 0, 2)
        giq_l[:, :, n0 : n0 + ch] = blk[:, 1].transpose(1, 0, 2)
        n0 += ch
    spq = np.ascontiguousarray(spq_l.transpose(2, 0, 1).reshape(NROWS, L))
    giq = np.ascontiguousarray(giq_l.transpose(2, 0, 1).reshape(NROWS, L))
    sp = spq.astype(np.float32)
    sp -= 127.5
    sp *= 1.0 / 127.5
    gini = giq.astype(np.float32)
    gini *= -0.5 / 255.0
    gini += 1.5
    return sp, gini


def _run(x, W, b, contribution, trace=False, tmpdir=None):
    from concourse import bass_utils

    nc = _get_module()

    x_flat = np.ascontiguousarray(x, dtype=np.float32).reshape(NCORES, NROWS, F)
    wt, w6 = _prep_wt(np.asarray(W, np.float32), np.asarray(b, np.float32))
    c = np.asarray(contribution, np.float32)
    d = np.ascontiguousarray(c[:, :, 0] - c[:, :, 1], dtype=np.float32)
    dr = np.ascontiguousarray(d.T.astype(np.float16).reshape(2, 128, T))

    with ThreadPoolExecutor(NCORES) as ex:
        xs = list(ex.map(_prep_core_x, [x_flat[i] for i in range(NCORES)]))

    if trace:
        _install_ntff_hook()
    in_maps = [
        {"xt": xs[i][0], "x6": xs[i][1], "wt": wt, "dr": dr, "w6": w6}
        for i in range(NCORES)
    ]
    res = bass_utils.run_bass_kernel_spmd(
        nc, in_maps, core_ids=list(range(NCORES)), trace=trace, tmpdir=tmpdir
    )

    with ThreadPoolExecutor(NCORES) as ex:
        outs = list(ex.map(lambda i: _unstage(res.results[i]["oq"]), range(NCORES)))
    sp = np.concatenate([o[0] for o in outs]).reshape(B, T, L)
    gini = np.concatenate([o[1] for o in outs]).reshape(B, T, L)
    out = (sp, gini)
    return (out, res) if trace else (out, None)


def kernel(x, W, b, contribution):
    out, _ = _run(x, W, b, contribution, trace=False)
    return out
